# revision 2
# baseline (speedup 1.0000x reference)
"""Trainium2 Bass kernel for LocalCrossCorrelationWithSmoothnessLoss.

Full inputs in, full output out. Internally: pure data-parallel over the
batch dim (B=8 -> 8 NeuronCores); each core computes partial sums for its
image; the host combines them into the three scalar losses.

Per-core pipeline (one 1024x1024 image pair + two flow channels):
  products   IJ = I*J (DVE), I2 = I^2, J2 = J^2 (ACT), bf16
  stage 1    H-direction 9-tap box conv as banded matmuls on the PE
             (band stationary, map moving, bf16, fp32 accumulate).
             Product maps use an 81-scaled band so the later combine is
             pure tensor-tensor work (81*S_IJ - S_I*S_J etc.).
  transpose  PE transpose per 120-row chunk (chunk-aligned, w-halo baked
             into the source free-dim offsets)
  stage 2    W-direction box conv, same banded matmuls on transposed maps
  combine    crossN = 81S_IJ - S_I*S_J, IvarN = 81S_II - S_I^2,
             JvarN = 81S_JJ - S_J^2   (all plain TT)
             cc = crossN^2 * exp(-ln(IvarN*JvarN))   (ln/exp on ACT, fp32)
             accumulated per-partition via STT accum_out
  smooth     sum(s^2) (ACT accum), lag products sum(s[w]s[w+1]) and
             sum(s[h]s[h+1]) (STT accum; row shift via SBUF->SBUF DMA).
             Edge-column/row corrections are computed on the host.

Output per core: 8 partial sums. Host assembles the losses in float64.
"""
import sys
import types
import numpy as np

sys.path.insert(0, "/opt/trn_rl_repo")

import ml_dtypes
import bass_rust
import concourse.bass as bass
import concourse.tile as tile
from concourse import mybir
from concourse import bass_utils
from concourse import tile_utils

F32 = mybir.dt.float32
F32R = mybir.dt.float32r
BF16 = mybir.dt.bfloat16
ALU = mybir.AluOpType
ACTF = mybir.ActivationFunctionType

H = 1024
W = 1024
PAD = 4
WIN = 81.0
ALPHA = 0.01
EPS = 1e-9
STRIDE = 120

# chunk table: (out_lo, out_n, in_lo, in_n)
CHUNKS = []
for _c in range((H + STRIDE - 1) // STRIDE):
    _olo = STRIDE * _c
    _on = min(STRIDE, H - _olo)
    _ilo = max(0, _olo - PAD)
    _ihi = min(H, _olo + _on + PAD)
    CHUNKS.append((_olo, _on, _ilo, _ihi - _ilo))
NCH = len(CHUNKS)

# allow using the full usable SBUF (tile_utils default is stale at 192K)
tile_utils.max_sbuf_usage = 206 * 1024

_nc_cache = {}


def _legalize_waits(nc, max_waits=1):
    """walrus here accepts only one sync-wait command per instruction;
    split extras onto same-engine NoOps placed just before."""
    ctr = 0
    for f in nc.m.functions:
        for bb in f.blocks:
            insts = bb.instructions
            i = 0
            while i < len(insts):
                ins = insts[i]
                si = ins.sync_info
                if si is None:
                    i += 1
                    continue
                w = list(si.on_wait)
                if len(w) <= max_waits:
                    i += 1
                    continue
                extra, keep = w[:-max_waits], w[-max_waits:]
                nops = []
                for j in range(0, len(extra), max_waits):
                    chunk = extra[j:j + max_waits]
                    nop = mybir.InstNoOp(name=f"I-wsplit-{ctr}", ins=[], outs=[])
                    ctr += 1
                    nop.engine = ins.engine
                    nop.sync_info = bass_rust.SyncInfo(on_wait=chunk, on_update=[])
                    nops.append(nop)
                ins.sync_info = bass_rust.SyncInfo(on_wait=keep,
                                                  on_update=list(si.on_update))
                insts[i:i] = nops
                i += len(nops) + 1


def _make_host_consts():
    """Band matrices (bf16), identity (bf16), ones (f32)."""
    def band(klo, kn, olo, on, scale):
        k = np.arange(klo, klo + kn)[:, None]
        m = np.arange(olo, olo + on)[None, :]
        return (np.abs(k - m) <= PAD).astype(np.float32) * scale

    bands = np.zeros((128, 4 * STRIDE), dtype=np.float32)
    # variant 0: first chunk (c=0), scale 1;  variant 1: first chunk, 81
    # variant 2: interior (c>=1), scale 1;    variant 3: interior, 81
    olo0, on0, ilo0, in0 = CHUNKS[0]
    bands[:in0, 0:on0] = band(ilo0, in0, olo0, on0, 1.0)
    bands[:in0, STRIDE:STRIDE + on0] = band(ilo0, in0, olo0, on0, 81.0)
    olo1, on1, ilo1, in1 = CHUNKS[1]
    bands[:in1, 2 * STRIDE:2 * STRIDE + on1] = band(ilo1, in1, olo1, on1, 1.0)
    bands[:in1, 3 * STRIDE:3 * STRIDE + on1] = band(ilo1, in1, olo1, on1, 81.0)
    bands_bf = bands.astype(ml_dtypes.bfloat16)
    ident_bf = np.eye(128, dtype=np.float32).astype(ml_dtypes.bfloat16)
    ones_f32 = np.ones((128, 1), dtype=np.float32)
    return bands_bf, bands, ident_bf, ones_f32


def _band_ap(bands_t, c, scaled):
    """AP into the packed bands tile for chunk c."""
    olo, on, ilo, inn = CHUNKS[c]
    if c == 0:
        v = 1 if scaled else 0
    else:
        v = 3 if scaled else 2
    return bands_t[0:inn, v * STRIDE:v * STRIDE + on]


def _build(nc):
    I_d = nc.dram_tensor("I", [H, W], F32, kind="ExternalInput").ap()
    J_d = nc.dram_tensor("J", [H, W], F32, kind="ExternalInput").ap()
    s0_d = nc.dram_tensor("s0", [H, W], F32, kind="ExternalInput").ap()
    s1_d = nc.dram_tensor("s1", [H, W], F32, kind="ExternalInput").ap()
    bands_d = nc.dram_tensor("bands", [128, 4 * STRIDE], BF16,
                             kind="ExternalInput").ap()
    bandsr_d = nc.dram_tensor("bandsr", [128, 4 * STRIDE], F32R,
                              kind="ExternalInput").ap()
    ident_d = nc.dram_tensor("ident", [128, 128], BF16,
                             kind="ExternalInput").ap()
    ones_d = nc.dram_tensor("ones", [128, 1], F32, kind="ExternalInput").ap()
    part_d = nc.dram_tensor("partials", [1, 68], F32,
                            kind="ExternalOutput").ap()

    from contextlib import ExitStack
    with tile.TileContext(nc) as tc, ExitStack() as ctx:
        consts = ctx.enter_context(tc.tile_pool(name="consts", bufs=1))
        inp = ctx.enter_context(tc.tile_pool(name="inp", bufs=2))
        prod = ctx.enter_context(tc.tile_pool(name="prod", bufs=2))
        smap = ctx.enter_context(tc.tile_pool(name="smap", bufs=1))
        tmap = ctx.enter_context(tc.tile_pool(name="tmap", bufs=2))
        ctmp = ctx.enter_context(tc.tile_pool(name="ctmp", bufs=2))
        spool = ctx.enter_context(tc.tile_pool(name="spool", bufs=2))
        accp = ctx.enter_context(tc.tile_pool(name="accp", bufs=1))
        psA = ctx.enter_context(tc.tile_pool(name="psA", bufs=2, space="PSUM"))
        ps2 = ctx.enter_context(tc.tile_pool(name="ps2", bufs=1, space="PSUM"))
        psF = ctx.enter_context(tc.tile_pool(name="psF", bufs=1, space="PSUM"))

        bands_t = consts.tile([128, 4 * STRIDE], BF16)
        bandsr_t = consts.tile([128, 4 * STRIDE], F32R)
        ident_t = consts.tile([128, 128], BF16)
        ones_t = consts.tile([128, 1], F32)
        nc.sync.dma_start(bands_t[:], bands_d)
        nc.sync.dma_start(bandsr_t[:], bandsr_d)
        nc.sync.dma_start(ident_t[:], ident_d)
        nc.sync.dma_start(ones_t[:], ones_d)

        # accumulators: accum_out OVERWRITES, so every accumulating
        # instruction gets its own column; host sums the groups.
        # cols 0-17: cc per (chunk,half); 18-33: lag_w; 34-49: lag_h;
        # 50-51: lag_h boundary; 52-67: s^2
        acc = accp.tile([128, 68], F32)
        nc.vector.memset(acc[:], 0.0)

        # ---------------- stage 1: H-conv -> S maps --------------------
        # S maps: per map 9 chunk tiles [out_n<=120, W] bf16, persistent
        MAPS = ("si", "sj", "sij", "sii", "sjj")
        s_tiles = {}
        for c, (olo, on, ilo, inn) in enumerate(CHUNKS):
            I_t = inp.tile([128, W], F32, tag="I_in")
            J_t = inp.tile([128, W], F32, tag="J_in")
            nc.sync.dma_start(I_t[0:inn, :], I_d[ilo:ilo + inn, :])
            nc.scalar.dma_start(J_t[0:inn, :], J_d[ilo:ilo + inn, :])

            sts = {}
            for name in MAPS:
                sts[name] = smap.tile([128, W], BF16, tag=f"S_{name}_{c}",
                                      name=f"S_{name}_{c}")
                s_tiles[(name, c)] = sts[name]
            for hw in range(2):
                wsl = slice(512 * hw, 512 * hw + 512)
                I_r = prod.tile([128, 512], F32R, tag="I_r")
                J_r = prod.tile([128, 512], F32R, tag="J_r")
                nc.vector.tensor_copy(I_r[0:inn, :], I_t[0:inn, wsl])
                nc.vector.tensor_copy(J_r[0:inn, :], J_t[0:inn, wsl])
                IJ_r = prod.tile([128, 512], F32R, tag="IJ_r")
                nc.vector.tensor_tensor(out=IJ_r[0:inn, :],
                                        in0=I_t[0:inn, wsl],
                                        in1=J_t[0:inn, wsl], op=ALU.mult)
                I2_r = prod.tile([128, 512], F32R, tag="I2_r")
                J2_r = prod.tile([128, 512], F32R, tag="J2_r")
                nc.scalar.square(I2_r[0:inn, :], I_t[0:inn, wsl])
                nc.scalar.square(J2_r[0:inn, :], J_t[0:inn, wsl])
                srcs = (I_r, J_r, IJ_r, I2_r, J2_r)
                for mi, name in enumerate(MAPS):
                    scaled = mi >= 2
                    p1 = psA.tile([128, 512], F32, tag="psA",
                                  padded_shape=[128, 512])
                    nc.tensor.matmul(p1[0:on, :],
                                     _band_ap(bandsr_t, c, scaled),
                                     srcs[mi][0:inn, :],
                                     start=True, stop=True)
                    if (c * 10 + mi * 2 + hw) % 2 == 0:
                        nc.vector.tensor_copy(sts[name][0:on, wsl],
                                              p1[0:on, :])
                    else:
                        nc.scalar.copy(sts[name][0:on, wsl], p1[0:on, :])

        # ------------- stage 2 per chunk: transpose, W-conv, combine ----
        for c2, (olo2, on2, ilo2, in2) in enumerate(CHUNKS):
            n = on2
            t_tiles = {}
            for mi, name in enumerate(MAPS):
                # transpose all 9 h'-segments into one bf16 psum bank
                pT = psA.tile([128, H], BF16, tag="psA", name="pT")
                for ch, (holo, hon, _, _) in enumerate(CHUNKS):
                    st = s_tiles[(name, ch)]
                    nc.tensor.matmul(
                        pT[0:in2, holo:holo + hon],
                        st[0:hon, ilo2:ilo2 + in2],
                        ident_t[0:hon, 0:hon],
                        is_transpose=True,
                        start=(ch == 0), stop=(ch == NCH - 1),
                        skip_group_check=True,
                    )
                tt = tmap.tile([128, H], BF16, tag=f"T_{name}")
                if (c2 * 5 + mi) % 2 == 0:
                    nc.vector.tensor_copy(tt[0:in2, :], pT[0:in2, :])
                else:
                    nc.scalar.copy(tt[0:in2, :], pT[0:in2, :])
                t_tiles[name] = tt

            for hw in range(2):
                hsl = slice(512 * hw, 512 * hw + 512)
                p2 = {}
                for mi, name in enumerate(MAPS):
                    p2[name] = ps2.tile([128, 512], F32, tag=f"p2_{name}", name=f"p2_{name}")
                    nc.tensor.matmul(p2[name][0:n, :],
                                     _band_ap(bands_t, c2, False),
                                     t_tiles[name][0:in2, hsl],
                                     start=True, stop=True)

                # combine in fp32, reading stage-2 psum directly
                # (max one PSUM operand per instruction)
                si_sb = ctmp.tile([128, 512], F32, tag="si_sb")
                nc.scalar.copy(si_sb[0:n, :], p2["si"][0:n, :])
                P = ctmp.tile([128, 512], F32, tag="P")
                nc.vector.tensor_tensor(out=P[0:n, :], in0=si_sb[0:n, :],
                                        in1=p2["sj"][0:n, :], op=ALU.mult)
                crossN = ctmp.tile([128, 512], F32, tag="crossN")
                nc.vector.tensor_tensor(out=crossN[0:n, :],
                                        in0=p2["sij"][0:n, :],
                                        in1=P[0:n, :], op=ALU.subtract)
                # PII = si^2 in-place over si_sb (si_sb dead afterwards)
                nc.scalar.square(si_sb[0:n, :], si_sb[0:n, :])
                IvarN = ctmp.tile([128, 512], F32, tag="IvarN")
                nc.vector.tensor_tensor(out=IvarN[0:n, :],
                                        in0=p2["sii"][0:n, :],
                                        in1=si_sb[0:n, :], op=ALU.subtract)
                PJJ = ctmp.tile([128, 512], F32, tag="PJJ")
                nc.scalar.square(PJJ[0:n, :], p2["sj"][0:n, :])
                JvarN = ctmp.tile([128, 512], F32, tag="JvarN")
                nc.vector.tensor_tensor(out=JvarN[0:n, :],
                                        in0=p2["sjj"][0:n, :],
                                        in1=PJJ[0:n, :], op=ALU.subtract)
                denom = ctmp.tile([128, 512], F32, tag="denom")
                nc.vector.tensor_tensor(out=denom[0:n, :], in0=IvarN[0:n, :],
                                        in1=JvarN[0:n, :], op=ALU.mult)
                # recip = exp(-ln(denom)), in-place
                nc.scalar.activation(denom[0:n, :], denom[0:n, :], ACTF.Ln)
                nc.scalar.activation(denom[0:n, :], denom[0:n, :], ACTF.Exp,
                                     scale=-1.0)
                # c2sq in-place over crossN
                nc.scalar.square(crossN[0:n, :], crossN[0:n, :])
                nc.vector.scalar_tensor_tensor(
                    out=crossN[0:n, :], in0=crossN[0:n, :], scalar=1.0,
                    in1=denom[0:n, :], op0=ALU.mult, op1=ALU.mult,
                    accum_out=acc[0:n, c2 * 2 + hw:c2 * 2 + hw + 1])

        # ---------------- smoothness over s0, s1 ------------------------
        for ch_i, s_d in enumerate((s0_d, s1_d)):
            for t in range(8):
                st = spool.tile([128, W], F32, tag="s_in")
                eng_d = nc.sync if t % 2 == 0 else nc.scalar
                eng_d.dma_start(st[:], s_d[128 * t:128 * (t + 1), :])
                # sum s^2 (output is junk; only the accumulator matters)
                s2o = spool.tile([128, W], F32, tag="junk")
                nc.scalar.activation(s2o[:], st[:], ACTF.Square,
                                     accum_out=acc[:, 52 + ch_i * 8 + t:
                                                   53 + ch_i * 8 + t])
                # lag_w: s[w]*s[w+1]
                lw = spool.tile([128, W], F32, tag="junk")
                nc.vector.scalar_tensor_tensor(
                    out=lw[:, 0:W - 1], in0=st[:, 1:W], scalar=1.0,
                    in1=st[:, 0:W - 1], op0=ALU.mult, op1=ALU.mult,
                    accum_out=acc[:, 18 + ch_i * 8 + t:19 + ch_i * 8 + t])
                # lag_h within tile: shift rows down via SBUF->SBUF DMA
                sh = spool.tile([128, W], F32, tag="sh")
                eng_d2 = nc.scalar if t % 2 == 0 else nc.sync
                eng_d2.dma_start(sh[0:127, :], st[1:128, :])
                lh = spool.tile([128, W], F32, tag="junk")
                nc.vector.scalar_tensor_tensor(
                    out=lh[0:127, :], in0=sh[0:127, :], scalar=1.0,
                    in1=st[0:127, :], op0=ALU.mult, op1=ALU.mult,
                    accum_out=acc[0:127, 34 + ch_i * 8 + t:
                                  35 + ch_i * 8 + t])

        # ---------------- final partition reduction ---------------------
        pF = psF.tile([1, 68], F32)
        nc.tensor.matmul(pF[:], ones_t[:], acc[:], start=True, stop=True)
        outt = accp.tile([1, 68], F32, tag="outt")
        nc.scalar.copy(outt[:], pF[:])
        nc.sync.dma_start(part_d, outt[:])

    return


def _get_nc():
    if "nc" not in _nc_cache:
        nc = bass.Bass("TRN2", target_bir_lowering=False, debug=False)
        _build(nc)
        _legalize_waits(nc)
        _nc_cache["nc"] = nc
    return _nc_cache["nc"]


def _make_in_maps(I, J, s):
    B = I.shape[0]
    bands_bf, bands_f32, ident_bf, ones_f32 = _make_host_consts()
    in_maps = []
    for b in range(B):
        in_maps.append({
            "I": np.ascontiguousarray(I[b, 0]),
            "J": np.ascontiguousarray(J[b, 0]),
            "s0": np.ascontiguousarray(s[b, 0]),
            "s1": np.ascontiguousarray(s[b, 1]),
            "bands": bands_bf,
            "bandsr": bands_f32,
            "ident": ident_bf,
            "ones": ones_f32,
        })
    return in_maps


def kernel(I, J, s, sum_filt):
    B = I.shape[0]
    assert I.shape == (B, 1, H, W) and s.shape == (B, 2, H, W)
    nc = _get_nc()
    in_maps = _make_in_maps(I, J, s)
    res = bass_utils.run_bass_kernel_spmd(nc, in_maps,
                                          core_ids=list(range(B)))
    parts = np.stack([res.results[b]["partials"][0] for b in range(B)])
    parts = parts.astype(np.float64)

    # host-side final assembly (float64)
    s64 = s.astype(np.float64)
    cc_sum = float(parts[:, 0:18].sum())
    lag_w = parts[:, 18:34].sum(axis=1)
    lag_h = parts[:, 34:52].sum(axis=1)
    s2 = parts[:, 52:68].sum(axis=1)

    # tile-boundary lag_h pairs (rows 127/128, 255/256, ...) per core
    rb = np.arange(127, H - 1, 128)
    lag_h = lag_h + (s64[:, :, rb, :] * s64[:, :, rb + 1, :]).sum(axis=(1, 2, 3))

    # edge corrections per core (both channels folded together)
    e_w = (s64[:, :, :, 0] ** 2).sum(axis=(1, 2)) + \
          (s64[:, :, :, -1] ** 2).sum(axis=(1, 2))
    e_h = (s64[:, :, 0, :] ** 2).sum(axis=(1, 2)) + \
          (s64[:, :, -1, :] ** 2).sum(axis=(1, 2))

    sum_dx2 = (2.0 * s2 - e_w - 2.0 * lag_w).sum()
    sum_dy2 = (2.0 * s2 - e_h - 2.0 * lag_h).sum()
    cnt = B * 2 * H * (W - 1)

    ncc_loss = -cc_sum / (B * H * W)
    smooth = 0.5 * (sum_dx2 / cnt + sum_dy2 / cnt) * ALPHA
    total = ncc_loss + smooth
    return np.array([total, ncc_loss, smooth], dtype=np.float32)



# revision 27
# speedup vs baseline: 1.6572x; 1.6572x over previous
"""Trainium2 Bass kernel for LocalCrossCorrelationWithSmoothnessLoss.

Full inputs in, full output out. Pure data-parallel over the batch dim
(B=8 -> 8 NeuronCores); each core computes partial sums for its image;
the host combines them into the three scalar losses.

Per-core pipeline (one 1024x1024 image pair + two flow channels):
  load       I, J, s loaded as bf16 via SWDGE cast-DMA (gpsimd) --
             spreads across all 16 SDMA engines and needs no DVE casts.
  products   IJ (DVE TT bf16 2x), I^2 / J^2 (ACT Square), bf16.
  stage 1    H-direction 9-tap box conv as banded matmuls on the PE
             (band stationary bf16, map moving bf16, fp32 accumulate).
             Product maps use an 81-scaled band so the later combine is
             pure tensor work (81*S_IJ - S_I*S_J etc.).
  transpose  PE transpose per 120-row chunk into a bf16 PSUM bank.
  stage 2    W-direction box conv, banded matmuls on transposed maps.
  combine    bf16, FD=1024 (both halves at once):
             crossN = 81*S_IJ - S_I*S_J, IvarN = 81*S_II - S_I^2,
             JvarN = 81*S_JJ - S_J^2, cc = crossN^2 * exp(-ln(denom+eps))
             accumulated per-partition via STT accum_out.
  smooth     dx: gpsimd shifted subtract + ACT Square accum.
             dy: difference-band matmul on PE (psum = s[h+1]-s[h]) +
             ACT Square accum from PSUM.  Tile-boundary dy rows are
             host-corrected.  No SBUF->SBUF shift DMAs.

Output per core: 57 partial sums. Host assembles the losses in float64.
"""
import sys
import numpy as np

sys.path.insert(0, "/opt/trn_rl_repo")

import ml_dtypes
import bass_rust
import concourse.bass as bass
import concourse.tile as tile
from concourse import mybir
from concourse import bass_utils
from concourse import tile_utils

F32 = mybir.dt.float32
BF16 = mybir.dt.bfloat16
ALU = mybir.AluOpType
ACTF = mybir.ActivationFunctionType

H = 1024
W = 1024
PAD = 4
WIN = 81.0
ALPHA = 0.01
EPS = 1e-9
EPS_N = EPS * WIN * WIN    # eps in the 81x-scaled domain
STRIDE = 120

# chunk table: (out_lo, out_n, in_lo, in_n)
CHUNKS = []
for _c in range((H + STRIDE - 1) // STRIDE):
    _olo = STRIDE * _c
    _on = min(STRIDE, H - _olo)
    _ilo = max(0, _olo - PAD)
    _ihi = min(H, _olo + _on + PAD)
    CHUNKS.append((_olo, _on, _ilo, _ihi - _ilo))
NCH = len(CHUNKS)

# accumulator column layout
COL_CC = 0            # 9 cols, one per w-chunk
COL_DX = COL_CC + NCH          # 16 cols, one per (ch, tile)
COL_DY = COL_DX + 16           # 32 cols, one per (ch, tile, half)
NACC = COL_DY + 32             # 57

# allow using the full usable SBUF (tile_utils default is stale at 192K)
tile_utils.max_sbuf_usage = 206 * 1024

_nc_cache = {}


def _legalize_waits(nc, max_waits=1):
    """walrus here accepts only one sync-wait command per instruction;
    split extras onto same-engine NoOps placed just before."""
    ctr = 0
    for f in nc.m.functions:
        for bb in f.blocks:
            insts = bb.instructions
            i = 0
            while i < len(insts):
                ins = insts[i]
                si = ins.sync_info
                if si is None:
                    i += 1
                    continue
                w = list(si.on_wait)
                if len(w) <= max_waits:
                    i += 1
                    continue
                extra, keep = w[:-max_waits], w[-max_waits:]
                nops = []
                for j in range(0, len(extra), max_waits):
                    chunk = extra[j:j + max_waits]
                    nop = mybir.InstNoOp(name=f"I-wsplit-{ctr}", ins=[], outs=[])
                    ctr += 1
                    nop.engine = ins.engine
                    nop.sync_info = bass_rust.SyncInfo(on_wait=chunk, on_update=[])
                    nops.append(nop)
                ins.sync_info = bass_rust.SyncInfo(on_wait=keep,
                                                  on_update=list(si.on_update))
                insts[i:i] = nops
                i += len(nops) + 1


def _make_host_consts():
    """Band matrices (bf16), identity (bf16), diff band (bf16), ones."""
    def band(klo, kn, olo, on, scale):
        k = np.arange(klo, klo + kn)[:, None]
        m = np.arange(olo, olo + on)[None, :]
        return (np.abs(k - m) <= PAD).astype(np.float32) * scale

    bands = np.zeros((128, 4 * STRIDE), dtype=np.float32)
    # variant 0: first chunk (c=0), scale 1;  variant 1: first chunk, 81
    # variant 2: interior (c>=1), scale 1;    variant 3: interior, 81
    olo0, on0, ilo0, in0 = CHUNKS[0]
    bands[:in0, 0:on0] = band(ilo0, in0, olo0, on0, 1.0)
    bands[:in0, STRIDE:STRIDE + on0] = band(ilo0, in0, olo0, on0, 81.0)
    olo1, on1, ilo1, in1 = CHUNKS[1]
    bands[:in1, 2 * STRIDE:2 * STRIDE + on1] = band(ilo1, in1, olo1, on1, 1.0)
    bands[:in1, 3 * STRIDE:3 * STRIDE + on1] = band(ilo1, in1, olo1, on1, 81.0)
    bands_bf = bands.astype(ml_dtypes.bfloat16)
    ident_bf = np.eye(128, dtype=np.float32).astype(ml_dtypes.bfloat16)
    # difference band: out[m] = s[m+1] - s[m], m in [0, 126]
    dband = np.zeros((128, 128), dtype=np.float32)
    for m in range(127):
        dband[m + 1, m] = 1.0
        dband[m, m] = -1.0
    dband_bf = dband.astype(ml_dtypes.bfloat16)
    ones_f32 = np.ones((128, 1), dtype=np.float32)
    return bands_bf, ident_bf, dband_bf, ones_f32


def _band_ap(bands_t, c, scaled):
    """AP into the packed bands tile for chunk c."""
    olo, on, ilo, inn = CHUNKS[c]
    if c == 0:
        v = 1 if scaled else 0
    else:
        v = 3 if scaled else 2
    return bands_t[0:inn, v * STRIDE:v * STRIDE + on]


def _build(nc):
    I_d = nc.dram_tensor("I", [H, W], F32, kind="ExternalInput").ap()
    J_d = nc.dram_tensor("J", [H, W], F32, kind="ExternalInput").ap()
    s0_d = nc.dram_tensor("s0", [H, W], F32, kind="ExternalInput").ap()
    s1_d = nc.dram_tensor("s1", [H, W], F32, kind="ExternalInput").ap()
    bands_d = nc.dram_tensor("bands", [128, 4 * STRIDE], BF16,
                             kind="ExternalInput").ap()
    ident_d = nc.dram_tensor("ident", [128, 128], BF16,
                             kind="ExternalInput").ap()
    dband_d = nc.dram_tensor("dband", [128, 128], BF16,
                             kind="ExternalInput").ap()
    MAPS = ("si", "sj", "sij", "sii", "sjj")
    ones_d = nc.dram_tensor("ones", [128, 1], F32, kind="ExternalInput").ap()
    part_d = nc.dram_tensor("partials", [1, NACC], F32,
                            kind="ExternalOutput").ap()

    from contextlib import ExitStack
    with tile.TileContext(nc) as tc, ExitStack() as ctx:
        consts = ctx.enter_context(tc.tile_pool(name="consts", bufs=1))
        inp = ctx.enter_context(tc.tile_pool(name="inp", bufs=2))
        prod = ctx.enter_context(tc.tile_pool(name="prod", bufs=2))
        smap = ctx.enter_context(tc.tile_pool(name="smap", bufs=1))
        tmap = ctx.enter_context(tc.tile_pool(name="tmap", bufs=2))
        comb = ctx.enter_context(tc.tile_pool(name="comb", bufs=2))
        spool = ctx.enter_context(tc.tile_pool(name="spool", bufs=2))
        accp = ctx.enter_context(tc.tile_pool(name="accp", bufs=1))
        psA = ctx.enter_context(tc.tile_pool(name="psA", bufs=2, space="PSUM"))
        pbig = ctx.enter_context(tc.tile_pool(name="pbig", bufs=3,
                                              space="PSUM"))

        bands_t = consts.tile([128, 4 * STRIDE], BF16)
        ident_t = consts.tile([128, 128], BF16)
        dband_t = consts.tile([128, 128], BF16)
        ones_t = consts.tile([128, 1], F32)
        nc.sync.dma_start(bands_t[:], bands_d)
        nc.sync.dma_start(ident_t[:], ident_d)
        nc.sync.dma_start(dband_t[:], dband_d)
        nc.sync.dma_start(ones_t[:], ones_d)

        # accumulators: accum_out OVERWRITES, so every accumulating
        # instruction gets its own column; host sums the groups.
        acc = accp.tile([128, NACC], F32)
        nc.vector.memset(acc[:], 0.0)
        epsb = consts.tile([128, 1], F32)
        nc.vector.memset(epsb[:], EPS_N)

        # ---------------- smoothness tile (interleaved) -----------------
        def smooth_tile(k):
            ch_i, t = k // 8, k % 8
            s_d = s0_d if ch_i == 0 else s1_d
            st = spool.tile([128, W], BF16, tag="s_in")
            nc.gpsimd.dma_start(st[:], s_d[128 * t:128 * (t + 1), :])
            # dx: shifted subtract (gpsimd) + square-accum (DVE/ACT alt)
            sub = spool.tile([128, W], BF16, tag="s_sub")
            nc.gpsimd.tensor_tensor(out=sub[:, 0:W - 1], in0=st[:, 1:W],
                                    in1=st[:, 0:W - 1], op=ALU.subtract)
            junk = spool.tile([128, W], BF16, tag="s_junk")
            cx = COL_DX + ch_i * 8 + t
            if k % 2 == 0:
                nc.scalar.activation(junk[:, 0:W - 1], sub[:, 0:W - 1],
                                     ACTF.Square,
                                     accum_out=acc[:, cx:cx + 1])
            else:
                nc.vector.scalar_tensor_tensor(
                    out=junk[:, 0:W - 1], in0=sub[:, 0:W - 1], scalar=1.0,
                    in1=sub[:, 0:W - 1], op0=ALU.mult, op1=ALU.mult,
                    accum_out=acc[:, cx:cx + 1])
            # dy: difference-band matmul (PE) then Square accum from PSUM
            for hw in range(2):
                hsl = slice(512 * hw, 512 * hw + 512)
                pD = psA.tile([128, 512], F32, tag="psA",
                              padded_shape=[128, 512])
                nc.tensor.matmul(pD[0:127, :], dband_t[0:128, 0:127],
                                 st[0:128, hsl], start=True, stop=True)
                col = COL_DY + (ch_i * 8 + t) * 2 + hw
                nc.scalar.activation(junk[0:127, hsl], pD[0:127, :],
                                     ACTF.Square,
                                     accum_out=acc[0:127, col:col + 1])

        # ---------------- stage 1: H-conv -> S maps --------------------
        s_tiles = {}
        for c, (olo, on, ilo, inn) in enumerate(CHUNKS):
            I_t = inp.tile([128, W], BF16, tag="I_in")
            J_t = inp.tile([128, W], BF16, tag="J_in")
            nc.gpsimd.dma_start(I_t[0:inn, :], I_d[ilo:ilo + inn, :])
            nc.gpsimd.dma_start(J_t[0:inn, :], J_d[ilo:ilo + inn, :])

            IJ_t = prod.tile([128, W], BF16, tag="IJ")
            I2_t = prod.tile([128, W], BF16, tag="I2")
            J2_t = prod.tile([128, W], BF16, tag="J2")
            nc.vector.tensor_tensor(out=IJ_t[0:inn, :], in0=I_t[0:inn, :],
                                    in1=J_t[0:inn, :], op=ALU.mult)
            nc.scalar.square(I2_t[0:inn, :], I_t[0:inn, :])
            nc.gpsimd.tensor_tensor(out=J2_t[0:inn, :], in0=J_t[0:inn, :],
                                    in1=J_t[0:inn, :], op=ALU.mult)

            srcs = (I_t, J_t, IJ_t, I2_t, J2_t)
            for mi, name in enumerate(MAPS):
                scaled = mi >= 2
                p1 = pbig.tile([128, W], F32, tag="pb", name=f"p1_{name}")
                for hw in range(2):
                    wsl = slice(512 * hw, 512 * hw + 512)
                    nc.tensor.matmul(p1[0:on, wsl],
                                     _band_ap(bands_t, c, scaled),
                                     srcs[mi][0:inn, wsl],
                                     start=True, stop=True)
                ssb = smap.tile([128, W], BF16, tag=f"S_{name}_{c}",
                                name=f"S_{name}_{c}")
                s_tiles[(name, c)] = ssb
                if (c * 5 + mi) % 2 == 0:
                    nc.vector.tensor_copy(ssb[0:on, :], p1[0:on, :])
                else:
                    nc.scalar.copy(ssb[0:on, :], p1[0:on, :])

            # smoothness tiles 0..8 fill stage-1 troughs
            smooth_tile(c)

        # ------------- stage 2 per chunk: transpose, W-conv, combine ----
        for c2, (olo2, on2, ilo2, in2) in enumerate(CHUNKS):
            n = on2
            t_tiles = {}

            def transpose_map(name, mi):
                # transpose via REGULAR matmuls (S.T @ ident): engages the
                # HAM clock boost and FWL, unlike transpose-mode.  Output
                # is f32 psum; pieces crossing the 512-col bank boundary
                # split so each matmul stays within one bank.
                pT = pbig.tile([128, H], F32, tag="pb", name="pT")
                for chk, (holo, hon, _, _) in enumerate(CHUNKS):
                    stc = s_tiles[(name, chk)]
                    segs = ([(holo, 512), (512, holo + hon)]
                            if holo < 512 < holo + hon
                            else [(holo, holo + hon)])
                    for lo, hi in segs:
                        nc.tensor.matmul(
                            pT[0:in2, lo:hi],
                            stc[0:hon, ilo2:ilo2 + in2],
                            ident_t[0:hon, lo - holo:hi - holo],
                            start=True, stop=True,
                        )
                tt = tmap.tile([128, H], BF16, tag=f"T_{name}")
                if (c2 * 5 + mi) % 2 == 0:
                    nc.vector.tensor_copy(tt[0:in2, :], pT[0:in2, :])
                else:
                    nc.scalar.copy(tt[0:in2, :], pT[0:in2, :])
                t_tiles[name] = tt

            def s2_matmul(name):
                # transpose this map right before its W-conv so dense
                # N=512 streams interleave the short transpose streams
                # (keeps the HAM clock-gate from re-throttling)
                p2 = pbig.tile([128, H], F32, tag="pb", name=f"p2_{name}")
                for hw in range(2):
                    hsl = slice(512 * hw, 512 * hw + 512)
                    nc.tensor.matmul(p2[0:n, hsl],
                                     _band_ap(bands_t, c2, False),
                                     t_tiles[name][0:in2, hsl],
                                     start=True, stop=True)
                return p2

            # si / sj: square from psum (ACT) + copy to SBUF, psum frees
            si = comb.tile([128, H], BF16, tag="si")
            sj = comb.tile([128, H], BF16, tag="sj")
            sqI = comb.tile([128, H], BF16, tag="sqI")
            sqJ = comb.tile([128, H], BF16, tag="sqJ")
            transpose_map("si", 0)
            p2si = s2_matmul("si")
            nc.scalar.square(sqI[0:n, :], p2si[0:n, :])
            nc.vector.tensor_copy(si[0:n, :], p2si[0:n, :])
            transpose_map("sj", 1)
            p2sj = s2_matmul("sj")
            nc.scalar.square(sqJ[0:n, :], p2sj[0:n, :])
            nc.scalar.copy(sj[0:n, :], p2sj[0:n, :])
            # P = si*sj on gpsimd (SBUF bf16)
            P = comb.tile([128, H], BF16, tag="P")
            nc.gpsimd.tensor_tensor(out=P[0:n, :], in0=si[0:n, :],
                                    in1=sj[0:n, :], op=ALU.mult)
            # single-use maps: combine reads psum directly (DVE TT 1x)
            crossN = comb.tile([128, H], BF16, tag="crossN")
            transpose_map("sij", 2)
            p2sij = s2_matmul("sij")
            nc.vector.tensor_tensor(out=crossN[0:n, :], in0=p2sij[0:n, :],
                                    in1=P[0:n, :], op=ALU.subtract)
            transpose_map("sii", 3)
            p2sii = s2_matmul("sii")
            nc.vector.tensor_tensor(out=sqI[0:n, :], in0=p2sii[0:n, :],
                                    in1=sqI[0:n, :], op=ALU.subtract)
            transpose_map("sjj", 4)
            p2sjj = s2_matmul("sjj")
            nc.vector.tensor_tensor(out=sqJ[0:n, :], in0=p2sjj[0:n, :],
                                    in1=sqJ[0:n, :], op=ALU.subtract)
            # denom over sqI (gpsimd); recip = exp(-ln(denom+eps)) in-place
            nc.gpsimd.tensor_tensor(out=sqI[0:n, :], in0=sqI[0:n, :],
                                    in1=sqJ[0:n, :], op=ALU.mult)
            nc.scalar.activation(sqI[0:n, :], sqI[0:n, :], ACTF.Ln,
                                 bias=epsb[0:n, 0:1])
            nc.scalar.activation(sqI[0:n, :], sqI[0:n, :], ACTF.Exp,
                                 scale=-1.0)
            # crossN^2 in-place, then accumulate cc
            nc.scalar.square(crossN[0:n, :], crossN[0:n, :])
            nc.vector.scalar_tensor_tensor(
                out=P[0:n, :], in0=crossN[0:n, :], scalar=1.0,
                in1=sqI[0:n, :], op0=ALU.mult, op1=ALU.mult,
                accum_out=acc[0:n, COL_CC + c2:COL_CC + c2 + 1])

            # smoothness tiles 9..15 fill stage-2 troughs
            if c2 < 7:
                smooth_tile(9 + c2)

        # ---------------- final partition reduction ---------------------
        pF = psA.tile([128, 512], F32, tag="psA", name="pF")
        nc.tensor.matmul(pF[0:1, 0:NACC], ones_t[:], acc[:],
                         start=True, stop=True)
        outt = accp.tile([1, NACC], F32, tag="outt")
        nc.scalar.copy(outt[:], pF[0:1, 0:NACC])
        nc.sync.dma_start(part_d, outt[:])

    return


# revision 30
# speedup vs baseline: 1.9801x; 1.1948x over previous
"""Trainium2 Bass kernel for LocalCrossCorrelationWithSmoothnessLoss.

Full inputs in, full output out. Pure data-parallel over the batch dim
(B=8 -> 8 NeuronCores); each core computes partial sums for its image;
the host combines them into the three scalar losses.

Per-core pipeline (one 1024x1024 image pair + two flow channels):
  load       I, J, s loaded as bf16 via SWDGE cast-DMA (gpsimd) --
             spreads across all 16 SDMA engines and needs no DVE casts.
  products   IJ (DVE TT bf16 2x), I^2 / J^2 (ACT Square), bf16.
  stage 1    H-direction 9-tap box conv as banded matmuls on the PE
             (band stationary bf16, map moving bf16, fp32 accumulate).
             Product maps use an 81-scaled band so the later combine is
             pure tensor work (81*S_IJ - S_I*S_J etc.).
  transpose  PE transpose per 120-row chunk into a bf16 PSUM bank.
  stage 2    W-direction box conv, banded matmuls on transposed maps.
  combine    bf16, FD=1024 (both halves at once):
             crossN = 81*S_IJ - S_I*S_J, IvarN = 81*S_II - S_I^2,
             JvarN = 81*S_JJ - S_J^2, cc = crossN^2 * exp(-ln(denom+eps))
             accumulated per-partition via STT accum_out.
  smooth     dx: gpsimd shifted subtract + ACT Square accum.
             dy: difference-band matmul on PE (psum = s[h+1]-s[h]) +
             ACT Square accum from PSUM.  Tile-boundary dy rows are
             host-corrected.  No SBUF->SBUF shift DMAs.

Output per core: 57 partial sums. Host assembles the losses in float64.
"""
import sys
import numpy as np

sys.path.insert(0, "/opt/trn_rl_repo")

import ml_dtypes
import bass_rust
import concourse.bass as bass
import concourse.tile as tile
from concourse import mybir
from concourse import bass_utils
from concourse import tile_utils

F32 = mybir.dt.float32
BF16 = mybir.dt.bfloat16
ALU = mybir.AluOpType
ACTF = mybir.ActivationFunctionType

H = 1024
W = 1024
PAD = 4
WIN = 81.0
ALPHA = 0.01
EPS = 1e-9
EPS_N = EPS * WIN * WIN    # eps in the 81x-scaled domain
STRIDE = 120

# chunk table: (out_lo, out_n, in_lo, in_n)
CHUNKS = []
for _c in range((H + STRIDE - 1) // STRIDE):
    _olo = STRIDE * _c
    _on = min(STRIDE, H - _olo)
    _ilo = max(0, _olo - PAD)
    _ihi = min(H, _olo + _on + PAD)
    CHUNKS.append((_olo, _on, _ilo, _ihi - _ilo))
NCH = len(CHUNKS)

# accumulator column layout
COL_CC = 0            # 9 cols, one per w-chunk
COL_DX = COL_CC + NCH          # 16 cols, one per (ch, tile)
COL_DY = COL_DX + 16           # 32 cols, one per (ch, tile, half)
NACC = COL_DY + 32             # 57

# allow using the full usable SBUF (tile_utils default is stale at 192K)
tile_utils.max_sbuf_usage = 206 * 1024

_nc_cache = {}


def _legalize_waits(nc, max_waits=1):
    """walrus here accepts only one sync-wait command per instruction;
    split extras onto same-engine NoOps placed just before."""
    ctr = 0
    for f in nc.m.functions:
        for bb in f.blocks:
            insts = bb.instructions
            i = 0
            while i < len(insts):
                ins = insts[i]
                si = ins.sync_info
                if si is None:
                    i += 1
                    continue
                w = list(si.on_wait)
                if len(w) <= max_waits:
                    i += 1
                    continue
                extra, keep = w[:-max_waits], w[-max_waits:]
                nops = []
                for j in range(0, len(extra), max_waits):
                    chunk = extra[j:j + max_waits]
                    nop = mybir.InstNoOp(name=f"I-wsplit-{ctr}", ins=[], outs=[])
                    ctr += 1
                    nop.engine = ins.engine
                    nop.sync_info = bass_rust.SyncInfo(on_wait=chunk, on_update=[])
                    nops.append(nop)
                ins.sync_info = bass_rust.SyncInfo(on_wait=keep,
                                                  on_update=list(si.on_update))
                insts[i:i] = nops
                i += len(nops) + 1


def _make_host_consts():
    """Band matrices (bf16), identity (bf16), diff band (bf16), ones."""
    def band(klo, kn, olo, on, scale):
        k = np.arange(klo, klo + kn)[:, None]
        m = np.arange(olo, olo + on)[None, :]
        return (np.abs(k - m) <= PAD).astype(np.float32) * scale

    bands = np.zeros((128, 4 * STRIDE), dtype=np.float32)
    # variant 0: first chunk (c=0), scale 1;  variant 1: first chunk, 81
    # variant 2: interior (c>=1), scale 1;    variant 3: interior, 81
    olo0, on0, ilo0, in0 = CHUNKS[0]
    bands[:in0, 0:on0] = band(ilo0, in0, olo0, on0, 1.0)
    bands[:in0, STRIDE:STRIDE + on0] = band(ilo0, in0, olo0, on0, 81.0)
    olo1, on1, ilo1, in1 = CHUNKS[1]
    bands[:in1, 2 * STRIDE:2 * STRIDE + on1] = band(ilo1, in1, olo1, on1, 1.0)
    bands[:in1, 3 * STRIDE:3 * STRIDE + on1] = band(ilo1, in1, olo1, on1, 81.0)
    bands_bf = bands.astype(ml_dtypes.bfloat16)
    ident_bf = np.eye(128, dtype=np.float32).astype(ml_dtypes.bfloat16)
    # difference band: out[m] = s[m+1] - s[m], m in [0, 126]
    dband = np.zeros((128, 128), dtype=np.float32)
    for m in range(127):
        dband[m + 1, m] = 1.0
        dband[m, m] = -1.0
    dband_bf = dband.astype(ml_dtypes.bfloat16)
    ones_f32 = np.ones((128, 1), dtype=np.float32)
    return bands_bf, ident_bf, dband_bf, ones_f32


def _band_ap(bands_t, c, scaled):
    """AP into the packed bands tile for chunk c."""
    olo, on, ilo, inn = CHUNKS[c]
    if c == 0:
        v = 1 if scaled else 0
    else:
        v = 3 if scaled else 2
    return bands_t[0:inn, v * STRIDE:v * STRIDE + on]


def _build(nc):
    I_d = nc.dram_tensor("I", [H, W], F32, kind="ExternalInput").ap()
    J_d = nc.dram_tensor("J", [H, W], F32, kind="ExternalInput").ap()
    s0_d = nc.dram_tensor("s0", [H, W], F32, kind="ExternalInput").ap()
    s1_d = nc.dram_tensor("s1", [H, W], F32, kind="ExternalInput").ap()
    bands_d = nc.dram_tensor("bands", [128, 4 * STRIDE], BF16,
                             kind="ExternalInput").ap()
    ident_d = nc.dram_tensor("ident", [128, 128], BF16,
                             kind="ExternalInput").ap()
    dband_d = nc.dram_tensor("dband", [128, 128], BF16,
                             kind="ExternalInput").ap()
    MAPS = ("si", "sj", "sij", "sii", "sjj")
    ones_d = nc.dram_tensor("ones", [128, 1], F32, kind="ExternalInput").ap()
    part_d = nc.dram_tensor("partials", [1, NACC], F32,
                            kind="ExternalOutput").ap()

    from contextlib import ExitStack
    with tile.TileContext(nc) as tc, ExitStack() as ctx:
        consts = ctx.enter_context(tc.tile_pool(name="consts", bufs=1))
        inp = ctx.enter_context(tc.tile_pool(name="inp", bufs=2))
        prod = ctx.enter_context(tc.tile_pool(name="prod", bufs=2))
        smap = ctx.enter_context(tc.tile_pool(name="smap", bufs=1))
        tmap = ctx.enter_context(tc.tile_pool(name="tmap", bufs=2))
        comb = ctx.enter_context(tc.tile_pool(name="comb", bufs=2))
        spool = ctx.enter_context(tc.tile_pool(name="spool", bufs=2))
        accp = ctx.enter_context(tc.tile_pool(name="accp", bufs=1))
        psA = ctx.enter_context(tc.tile_pool(name="psA", bufs=2, space="PSUM"))
        pbig = ctx.enter_context(tc.tile_pool(name="pbig", bufs=3,
                                              space="PSUM"))

        bands_t = consts.tile([128, 4 * STRIDE], BF16)
        ident_t = consts.tile([128, 128], BF16)
        dband_t = consts.tile([128, 128], BF16)
        ones_t = consts.tile([128, 1], F32)
        nc.sync.dma_start(bands_t[:], bands_d)
        nc.sync.dma_start(ident_t[:], ident_d)
        nc.sync.dma_start(dband_t[:], dband_d)
        nc.sync.dma_start(ones_t[:], ones_d)

        # accumulators: accum_out OVERWRITES, so every accumulating
        # instruction gets its own column; host sums the groups.
        acc = accp.tile([128, NACC], F32)
        nc.vector.memset(acc[:], 0.0)
        epsb = consts.tile([128, 1], F32)
        nc.vector.memset(epsb[:], EPS_N)

        # ---------------- smoothness tile (interleaved) -----------------
        # dx/dy via shifted SECOND loads from HBM (aligned bf16 -> DVE
        # 2x-mode subtract), squares accumulated on ACT.  No PE work.
        def smooth_tile(k):
            ch_i, t = k // 8, k % 8
            s_d = s0_d if ch_i == 0 else s1_d
            st = spool.tile([128, W], BF16, tag="s_in")
            nc.gpsimd.dma_start(st[:], s_d[128 * t:128 * (t + 1), :])
            # dx: column-shifted load, aligned subtract
            stx = spool.tile([128, W], BF16, tag="s_inx")
            nc.gpsimd.dma_start(stx[:, 0:W - 1],
                                s_d[128 * t:128 * (t + 1), 1:W])
            sub = spool.tile([128, W], BF16, tag="s_sub")
            nc.vector.tensor_tensor(out=sub[:, 0:W - 1], in0=stx[:, 0:W - 1],
                                    in1=st[:, 0:W - 1], op=ALU.subtract)
            junk = spool.tile([128, W], BF16, tag="s_junk")
            cx = COL_DX + ch_i * 8 + t
            nc.scalar.activation(junk[:, 0:W - 1], sub[:, 0:W - 1],
                                 ACTF.Square,
                                 accum_out=acc[:, cx:cx + 1])
            # dy: row-shifted load, aligned subtract (row 127 of the last
            # tile and tile boundaries are host-corrected)
            nrow = 127 if t == 7 else 128
            sty = spool.tile([128, W], BF16, tag="s_iny")
            nc.gpsimd.dma_start(sty[0:nrow, :],
                                s_d[128 * t + 1:128 * t + 1 + nrow, :])
            suby = spool.tile([128, W], BF16, tag="s_suby")
            nc.vector.tensor_tensor(out=suby[0:127, :], in0=sty[0:127, :],
                                    in1=st[0:127, :], op=ALU.subtract)
            col = COL_DY + (ch_i * 8 + t) * 2
            nc.scalar.activation(junk[0:127, :], suby[0:127, :],
                                 ACTF.Square,
                                 accum_out=acc[0:127, col:col + 1])

        # ---------------- stage 1: H-conv -> S maps --------------------
        s_tiles = {}
        for c, (olo, on, ilo, inn) in enumerate(CHUNKS):
            I_t = inp.tile([128, W], BF16, tag="I_in")
            J_t = inp.tile([128, W], BF16, tag="J_in")
            nc.gpsimd.dma_start(I_t[0:inn, :], I_d[ilo:ilo + inn, :])
            nc.gpsimd.dma_start(J_t[0:inn, :], J_d[ilo:ilo + inn, :])

            IJ_t = prod.tile([128, W], BF16, tag="IJ")
            I2_t = prod.tile([128, W], BF16, tag="I2")
            J2_t = prod.tile([128, W], BF16, tag="J2")
            nc.vector.tensor_tensor(out=IJ_t[0:inn, :], in0=I_t[0:inn, :],
                                    in1=J_t[0:inn, :], op=ALU.mult)
            nc.scalar.square(I2_t[0:inn, :], I_t[0:inn, :])
            nc.gpsimd.tensor_tensor(out=J2_t[0:inn, :], in0=J_t[0:inn, :],
                                    in1=J_t[0:inn, :], op=ALU.mult)

            srcs = (I_t, J_t, IJ_t, I2_t, J2_t)
            for mi, name in enumerate(MAPS):
                scaled = mi >= 2
                p1 = pbig.tile([128, W], F32, tag="pb", name=f"p1_{name}")
                for hw in range(2):
                    wsl = slice(512 * hw, 512 * hw + 512)
                    nc.tensor.matmul(p1[0:on, wsl],
                                     _band_ap(bands_t, c, scaled),
                                     srcs[mi][0:inn, wsl],
                                     start=True, stop=True)
                ssb = smap.tile([128, W], BF16, tag=f"S_{name}_{c}",
                                name=f"S_{name}_{c}")
                s_tiles[(name, c)] = ssb
                if (c * 5 + mi) % 2 == 0:
                    nc.vector.tensor_copy(ssb[0:on, :], p1[0:on, :])
                else:
                    nc.scalar.copy(ssb[0:on, :], p1[0:on, :])

            # smoothness tiles 0..8 fill stage-1 troughs
            smooth_tile(c)

        # ------------- stage 2 per chunk: transpose, W-conv, combine ----
        for c2, (olo2, on2, ilo2, in2) in enumerate(CHUNKS):
            n = on2
            t_tiles = {}
            for mi, name in enumerate(MAPS):
                # transpose via REGULAR matmuls (S.T @ ident): engages the
                # HAM clock boost and FWL, unlike transpose-mode.  Output
                # is f32 psum; pieces crossing the 512-col bank boundary
                # split so each matmul stays within one bank.
                pT = pbig.tile([128, H], F32, tag="pb", name="pT")
                for chk, (holo, hon, _, _) in enumerate(CHUNKS):
                    stc = s_tiles[(name, chk)]
                    segs = ([(holo, 512), (512, holo + hon)]
                            if holo < 512 < holo + hon
                            else [(holo, holo + hon)])
                    for lo, hi in segs:
                        nc.tensor.matmul(
                            pT[0:in2, lo:hi],
                            stc[0:hon, ilo2:ilo2 + in2],
                            ident_t[0:hon, lo - holo:hi - holo],
                            start=True, stop=True,
                        )
                tt = tmap.tile([128, H], BF16, tag=f"T_{name}")
                if (c2 * 5 + mi) % 2 == 0:
                    nc.vector.tensor_copy(tt[0:in2, :], pT[0:in2, :])
                else:
                    nc.scalar.copy(tt[0:in2, :], pT[0:in2, :])
                t_tiles[name] = tt

            def s2_matmul(name):
                p2 = pbig.tile([128, H], F32, tag="pb", name=f"p2_{name}")
                for hw in range(2):
                    hsl = slice(512 * hw, 512 * hw + 512)
                    nc.tensor.matmul(p2[0:n, hsl],
                                     _band_ap(bands_t, c2, False),
                                     t_tiles[name][0:in2, hsl],
                                     start=True, stop=True)
                return p2

            # si / sj: square from psum (ACT) + copy to SBUF, psum frees
            si = comb.tile([128, H], BF16, tag="si")
            sj = comb.tile([128, H], BF16, tag="sj")
            sqI = comb.tile([128, H], BF16, tag="sqI")
            sqJ = comb.tile([128, H], BF16, tag="sqJ")
            p2si = s2_matmul("si")
            nc.scalar.square(sqI[0:n, :], p2si[0:n, :])
            nc.vector.tensor_copy(si[0:n, :], p2si[0:n, :])
            p2sj = s2_matmul("sj")
            nc.scalar.square(sqJ[0:n, :], p2sj[0:n, :])
            nc.scalar.copy(sj[0:n, :], p2sj[0:n, :])
            # P = si*sj on gpsimd (SBUF bf16)
            P = comb.tile([128, H], BF16, tag="P")
            nc.gpsimd.tensor_tensor(out=P[0:n, :], in0=si[0:n, :],
                                    in1=sj[0:n, :], op=ALU.mult)
            # single-use maps: combine reads psum directly (DVE TT 1x)
            crossN = comb.tile([128, H], BF16, tag="crossN")
            p2sij = s2_matmul("sij")
            nc.vector.tensor_tensor(out=crossN[0:n, :], in0=p2sij[0:n, :],
                                    in1=P[0:n, :], op=ALU.subtract)
            p2sii = s2_matmul("sii")
            nc.vector.tensor_tensor(out=sqI[0:n, :], in0=p2sii[0:n, :],
                                    in1=sqI[0:n, :], op=ALU.subtract)
            p2sjj = s2_matmul("sjj")
            nc.vector.tensor_tensor(out=sqJ[0:n, :], in0=p2sjj[0:n, :],
                                    in1=sqJ[0:n, :], op=ALU.subtract)
            # denom over sqI (gpsimd); recip = exp(-ln(denom+eps)) in-place
            nc.gpsimd.tensor_tensor(out=sqI[0:n, :], in0=sqI[0:n, :],
                                    in1=sqJ[0:n, :], op=ALU.mult)
            nc.scalar.activation(sqI[0:n, :], sqI[0:n, :], ACTF.Ln,
                                 bias=epsb[0:n, 0:1])
            nc.scalar.activation(sqI[0:n, :], sqI[0:n, :], ACTF.Exp,
                                 scale=-1.0)
            # crossN^2 in-place, then accumulate cc
            nc.scalar.square(crossN[0:n, :], crossN[0:n, :])
            nc.vector.scalar_tensor_tensor(
                out=P[0:n, :], in0=crossN[0:n, :], scalar=1.0,
                in1=sqI[0:n, :], op0=ALU.mult, op1=ALU.mult,
                accum_out=acc[0:n, COL_CC + c2:COL_CC + c2 + 1])

            # smoothness tiles 9..15 fill stage-2 troughs
            if c2 < 7:
                smooth_tile(9 + c2)

        # ---------------- final partition reduction ---------------------
        pF = psA.tile([128, 512], F32, tag="psA", name="pF")
        nc.tensor.matmul(pF[0:1, 0:NACC], ones_t[:], acc[:],
                         start=True, stop=True)
        outt = accp.tile([1, NACC], F32, tag="outt")
        nc.scalar.copy(outt[:], pF[0:1, 0:NACC])
        nc.sync.dma_start(part_d, outt[:])

    return


# revision 31
# speedup vs baseline: 2.2419x; 1.1322x over previous
"""Trainium2 Bass kernel for LocalCrossCorrelationWithSmoothnessLoss.

Full inputs in, full output out. Pure data-parallel over the batch dim
(B=8 -> 8 NeuronCores); each core computes partial sums for its image;
the host combines them into the three scalar losses.

Per-core pipeline (one 1024x1024 image pair + two flow channels):
  load       I, J, s loaded as bf16 via SWDGE cast-DMA (gpsimd) --
             spreads across all 16 SDMA engines and needs no DVE casts.
  products   IJ (DVE TT bf16 2x), I^2 / J^2 (ACT Square), bf16.
  stage 1    H-direction 9-tap box conv as banded matmuls on the PE
             (band stationary bf16, map moving bf16, fp32 accumulate).
             Product maps use an 81-scaled band so the later combine is
             pure tensor work (81*S_IJ - S_I*S_J etc.).
  transpose  PE transpose per 120-row chunk into a bf16 PSUM bank.
  stage 2    W-direction box conv, banded matmuls on transposed maps.
  combine    bf16, FD=1024 (both halves at once):
             crossN = 81*S_IJ - S_I*S_J, IvarN = 81*S_II - S_I^2,
             JvarN = 81*S_JJ - S_J^2, cc = crossN^2 * exp(-ln(denom+eps))
             accumulated per-partition via STT accum_out.
  smooth     dx: gpsimd shifted subtract + ACT Square accum.
             dy: difference-band matmul on PE (psum = s[h+1]-s[h]) +
             ACT Square accum from PSUM.  Tile-boundary dy rows are
             host-corrected.  No SBUF->SBUF shift DMAs.

Output per core: 57 partial sums. Host assembles the losses in float64.
"""
import sys
import numpy as np

sys.path.insert(0, "/opt/trn_rl_repo")

import ml_dtypes
import bass_rust
import concourse.bass as bass
import concourse.tile as tile
from concourse import mybir
from concourse import bass_utils
from concourse import tile_utils

F32 = mybir.dt.float32
BF16 = mybir.dt.bfloat16
ALU = mybir.AluOpType
ACTF = mybir.ActivationFunctionType

H = 1024
W = 1024
PAD = 4
WIN = 81.0
ALPHA = 0.01
EPS = 1e-9
EPS_N = EPS * WIN * WIN    # eps in the 81x-scaled domain
STRIDE = 120

# chunk table: (out_lo, out_n, in_lo, in_n)
CHUNKS = []
for _c in range((H + STRIDE - 1) // STRIDE):
    _olo = STRIDE * _c
    _on = min(STRIDE, H - _olo)
    _ilo = max(0, _olo - PAD)
    _ihi = min(H, _olo + _on + PAD)
    CHUNKS.append((_olo, _on, _ilo, _ihi - _ilo))
NCH = len(CHUNKS)

# accumulator column layout
COL_CC = 0            # 9 cols, one per w-chunk
COL_DX = COL_CC + NCH          # 16 cols, one per (ch, tile)
COL_DY = COL_DX + 16           # 32 cols, one per (ch, tile, half)
NACC = COL_DY + 32             # 57

# allow using the full usable SBUF (tile_utils default is stale at 192K)
tile_utils.max_sbuf_usage = 206 * 1024

_nc_cache = {}


def _legalize_waits(nc, max_waits=1):
    """walrus here accepts only one sync-wait command per instruction;
    split extras onto same-engine NoOps placed just before."""
    ctr = 0
    for f in nc.m.functions:
        for bb in f.blocks:
            insts = bb.instructions
            i = 0
            while i < len(insts):
                ins = insts[i]
                si = ins.sync_info
                if si is None:
                    i += 1
                    continue
                w = list(si.on_wait)
                if len(w) <= max_waits:
                    i += 1
                    continue
                extra, keep = w[:-max_waits], w[-max_waits:]
                nops = []
                for j in range(0, len(extra), max_waits):
                    chunk = extra[j:j + max_waits]
                    nop = mybir.InstNoOp(name=f"I-wsplit-{ctr}", ins=[], outs=[])
                    ctr += 1
                    nop.engine = ins.engine
                    nop.sync_info = bass_rust.SyncInfo(on_wait=chunk, on_update=[])
                    nops.append(nop)
                ins.sync_info = bass_rust.SyncInfo(on_wait=keep,
                                                  on_update=list(si.on_update))
                insts[i:i] = nops
                i += len(nops) + 1


def _make_host_consts():
    """Band matrices (bf16), identity (bf16), diff band (bf16), ones."""
    def band(klo, kn, olo, on, scale):
        k = np.arange(klo, klo + kn)[:, None]
        m = np.arange(olo, olo + on)[None, :]
        return (np.abs(k - m) <= PAD).astype(np.float32) * scale

    bands = np.zeros((128, 4 * STRIDE), dtype=np.float32)
    # variant 0: first chunk (c=0), scale 1;  variant 1: first chunk, 81
    # variant 2: interior (c>=1), scale 1;    variant 3: interior, 81
    olo0, on0, ilo0, in0 = CHUNKS[0]
    bands[:in0, 0:on0] = band(ilo0, in0, olo0, on0, 1.0)
    bands[:in0, STRIDE:STRIDE + on0] = band(ilo0, in0, olo0, on0, 81.0)
    olo1, on1, ilo1, in1 = CHUNKS[1]
    bands[:in1, 2 * STRIDE:2 * STRIDE + on1] = band(ilo1, in1, olo1, on1, 1.0)
    bands[:in1, 3 * STRIDE:3 * STRIDE + on1] = band(ilo1, in1, olo1, on1, 81.0)
    bands_bf = bands.astype(ml_dtypes.bfloat16)
    ident_bf = np.eye(128, dtype=np.float32).astype(ml_dtypes.bfloat16)
    # difference band: out[m] = s[m+1] - s[m], m in [0, 126]
    dband = np.zeros((128, 128), dtype=np.float32)
    for m in range(127):
        dband[m + 1, m] = 1.0
        dband[m, m] = -1.0
    dband_bf = dband.astype(ml_dtypes.bfloat16)
    ones_f32 = np.ones((128, 1), dtype=np.float32)
    return bands_bf, ident_bf, dband_bf, ones_f32


def _band_ap(bands_t, c, scaled):
    """AP into the packed bands tile for chunk c."""
    olo, on, ilo, inn = CHUNKS[c]
    if c == 0:
        v = 1 if scaled else 0
    else:
        v = 3 if scaled else 2
    return bands_t[0:inn, v * STRIDE:v * STRIDE + on]


def _build(nc):
    I_d = nc.dram_tensor("I", [H, W], F32, kind="ExternalInput").ap()
    J_d = nc.dram_tensor("J", [H, W], F32, kind="ExternalInput").ap()
    s0_d = nc.dram_tensor("s0", [H, W], F32, kind="ExternalInput").ap()
    s1_d = nc.dram_tensor("s1", [H, W], F32, kind="ExternalInput").ap()
    bands_d = nc.dram_tensor("bands", [128, 4 * STRIDE], BF16,
                             kind="ExternalInput").ap()
    ident_d = nc.dram_tensor("ident", [128, 128], BF16,
                             kind="ExternalInput").ap()
    dband_d = nc.dram_tensor("dband", [128, 128], BF16,
                             kind="ExternalInput").ap()
    MAPS = ("si", "sj", "sij", "sii", "sjj")
    ones_d = nc.dram_tensor("ones", [128, 1], F32, kind="ExternalInput").ap()
    part_d = nc.dram_tensor("partials", [1, NACC], F32,
                            kind="ExternalOutput").ap()

    from contextlib import ExitStack
    with tile.TileContext(nc) as tc, ExitStack() as ctx:
        consts = ctx.enter_context(tc.tile_pool(name="consts", bufs=1))
        inp = ctx.enter_context(tc.tile_pool(name="inp", bufs=2))
        prod = ctx.enter_context(tc.tile_pool(name="prod", bufs=2))
        smap = ctx.enter_context(tc.tile_pool(name="smap", bufs=1))
        tmap = ctx.enter_context(tc.tile_pool(name="tmap", bufs=2))
        comb = ctx.enter_context(tc.tile_pool(name="comb", bufs=2))
        spool = ctx.enter_context(tc.tile_pool(name="spool", bufs=2))
        accp = ctx.enter_context(tc.tile_pool(name="accp", bufs=1))
        psA = ctx.enter_context(tc.tile_pool(name="psA", bufs=2, space="PSUM"))
        pbig = ctx.enter_context(tc.tile_pool(name="pbig", bufs=3,
                                              space="PSUM"))

        bands_t = consts.tile([128, 4 * STRIDE], BF16)
        ident_t = consts.tile([128, 128], BF16)
        dband_t = consts.tile([128, 128], BF16)
        ones_t = consts.tile([128, 1], F32)
        nc.sync.dma_start(bands_t[:], bands_d)
        nc.sync.dma_start(ident_t[:], ident_d)
        nc.sync.dma_start(dband_t[:], dband_d)
        nc.sync.dma_start(ones_t[:], ones_d)

        # accumulators: accum_out OVERWRITES, so every accumulating
        # instruction gets its own column; host sums the groups.
        acc = accp.tile([128, NACC], F32)
        nc.vector.memset(acc[:], 0.0)
        epsb = consts.tile([128, 1], F32)
        nc.vector.memset(epsb[:], EPS_N)

        # ---------------- smoothness tile (interleaved) -----------------
        def smooth_tile(k):
            ch_i, t = k // 8, k % 8
            s_d = s0_d if ch_i == 0 else s1_d
            st = spool.tile([128, W], BF16, tag="s_in")
            nc.gpsimd.dma_start(st[:], s_d[128 * t:128 * (t + 1), :])
            # dx: shifted subtract (gpsimd) + square-accum (DVE/ACT alt)
            sub = spool.tile([128, W], BF16, tag="s_sub")
            nc.gpsimd.tensor_tensor(out=sub[:, 0:W - 1], in0=st[:, 1:W],
                                    in1=st[:, 0:W - 1], op=ALU.subtract)
            junk = spool.tile([128, W], BF16, tag="s_junk")
            cx = COL_DX + ch_i * 8 + t
            if k % 2 == 0:
                nc.scalar.activation(junk[:, 0:W - 1], sub[:, 0:W - 1],
                                     ACTF.Square,
                                     accum_out=acc[:, cx:cx + 1])
            else:
                nc.vector.scalar_tensor_tensor(
                    out=junk[:, 0:W - 1], in0=sub[:, 0:W - 1], scalar=1.0,
                    in1=sub[:, 0:W - 1], op0=ALU.mult, op1=ALU.mult,
                    accum_out=acc[:, cx:cx + 1])
            # dy: difference-band matmul (PE) then Square accum from PSUM
            for hw in range(2):
                hsl = slice(512 * hw, 512 * hw + 512)
                pD = psA.tile([128, 512], F32, tag="psA",
                              padded_shape=[128, 512])
                nc.tensor.matmul(pD[0:127, :], dband_t[0:128, 0:127],
                                 st[0:128, hsl], start=True, stop=True)
                col = COL_DY + (ch_i * 8 + t) * 2 + hw
                nc.scalar.activation(junk[0:127, hsl], pD[0:127, :],
                                     ACTF.Square,
                                     accum_out=acc[0:127, col:col + 1])

        # ---------------- stage 1: H-conv -> S maps --------------------
        s_tiles = {}
        for c, (olo, on, ilo, inn) in enumerate(CHUNKS):
            I_t = inp.tile([128, W], BF16, tag="I_in")
            J_t = inp.tile([128, W], BF16, tag="J_in")
            nc.gpsimd.dma_start(I_t[0:inn, :], I_d[ilo:ilo + inn, :])
            nc.gpsimd.dma_start(J_t[0:inn, :], J_d[ilo:ilo + inn, :])

            IJ_t = prod.tile([128, W], BF16, tag="IJ")
            I2_t = prod.tile([128, W], BF16, tag="I2")
            J2_t = prod.tile([128, W], BF16, tag="J2")
            nc.vector.tensor_tensor(out=IJ_t[0:inn, :], in0=I_t[0:inn, :],
                                    in1=J_t[0:inn, :], op=ALU.mult)
            nc.scalar.square(I2_t[0:inn, :], I_t[0:inn, :])
            nc.gpsimd.tensor_tensor(out=J2_t[0:inn, :], in0=J_t[0:inn, :],
                                    in1=J_t[0:inn, :], op=ALU.mult)

            srcs = (I_t, J_t, IJ_t, I2_t, J2_t)
            for mi, name in enumerate(MAPS):
                scaled = mi >= 2
                p1 = pbig.tile([128, W], F32, tag="pb", name=f"p1_{name}")
                for hw in range(2):
                    wsl = slice(512 * hw, 512 * hw + 512)
                    nc.tensor.matmul(p1[0:on, wsl],
                                     _band_ap(bands_t, c, scaled),
                                     srcs[mi][0:inn, wsl],
                                     start=True, stop=True)
                ssb = smap.tile([128, W], BF16, tag=f"S_{name}_{c}",
                                name=f"S_{name}_{c}")
                s_tiles[(name, c)] = ssb
                if (c * 5 + mi) % 2 == 0:
                    nc.vector.tensor_copy(ssb[0:on, :], p1[0:on, :])
                else:
                    nc.scalar.copy(ssb[0:on, :], p1[0:on, :])

            # smoothness tiles 0..8 fill stage-1 troughs
            smooth_tile(c)

        # ------------- stage 2 per chunk: transpose, W-conv, combine ----
        for c2, (olo2, on2, ilo2, in2) in enumerate(CHUNKS):
            n = on2
            t_tiles = {}
            for mi, name in enumerate(MAPS):
                # transpose via REGULAR matmuls (S.T @ ident): engages the
                # HAM clock boost and FWL, unlike transpose-mode.  Output
                # is f32 psum; pieces crossing the 512-col bank boundary
                # split so each matmul stays within one bank.
                pT = pbig.tile([128, H], F32, tag="pb", name="pT")
                for chk, (holo, hon, _, _) in enumerate(CHUNKS):
                    stc = s_tiles[(name, chk)]
                    segs = ([(holo, 512), (512, holo + hon)]
                            if holo < 512 < holo + hon
                            else [(holo, holo + hon)])
                    for lo, hi in segs:
                        nc.tensor.matmul(
                            pT[0:in2, lo:hi],
                            stc[0:hon, ilo2:ilo2 + in2],
                            ident_t[0:hon, lo - holo:hi - holo],
                            start=True, stop=True,
                        )
                tt = tmap.tile([128, H], BF16, tag=f"T_{name}")
                if (c2 * 5 + mi) % 2 == 0:
                    nc.vector.tensor_copy(tt[0:in2, :], pT[0:in2, :])
                else:
                    nc.scalar.copy(tt[0:in2, :], pT[0:in2, :])
                t_tiles[name] = tt

            def s2_matmul(name):
                p2 = pbig.tile([128, H], F32, tag="pb", name=f"p2_{name}")
                for hw in range(2):
                    hsl = slice(512 * hw, 512 * hw + 512)
                    nc.tensor.matmul(p2[0:n, hsl],
                                     _band_ap(bands_t, c2, False),
                                     t_tiles[name][0:in2, hsl],
                                     start=True, stop=True)
                return p2

            # si / sj: square from psum (ACT) + copy to SBUF, psum frees
            si = comb.tile([128, H], BF16, tag="si")
            sj = comb.tile([128, H], BF16, tag="sj")
            sqI = comb.tile([128, H], BF16, tag="sqI")
            sqJ = comb.tile([128, H], BF16, tag="sqJ")
            p2si = s2_matmul("si")
            nc.scalar.square(sqI[0:n, :], p2si[0:n, :])
            nc.vector.tensor_copy(si[0:n, :], p2si[0:n, :])
            p2sj = s2_matmul("sj")
            nc.scalar.square(sqJ[0:n, :], p2sj[0:n, :])
            nc.scalar.copy(sj[0:n, :], p2sj[0:n, :])
            # P = si*sj on gpsimd (SBUF bf16)
            P = comb.tile([128, H], BF16, tag="P")
            nc.gpsimd.tensor_tensor(out=P[0:n, :], in0=si[0:n, :],
                                    in1=sj[0:n, :], op=ALU.mult)
            # single-use maps: combine reads psum directly (DVE TT 1x)
            crossN = comb.tile([128, H], BF16, tag="crossN")
            p2sij = s2_matmul("sij")
            nc.vector.tensor_tensor(out=crossN[0:n, :], in0=p2sij[0:n, :],
                                    in1=P[0:n, :], op=ALU.subtract)
            p2sii = s2_matmul("sii")
            nc.vector.tensor_tensor(out=sqI[0:n, :], in0=p2sii[0:n, :],
                                    in1=sqI[0:n, :], op=ALU.subtract)
            p2sjj = s2_matmul("sjj")
            nc.vector.tensor_tensor(out=sqJ[0:n, :], in0=p2sjj[0:n, :],
                                    in1=sqJ[0:n, :], op=ALU.subtract)
            # denom over sqI (gpsimd); recip = exp(-ln(denom+eps)) in-place
            nc.gpsimd.tensor_tensor(out=sqI[0:n, :], in0=sqI[0:n, :],
                                    in1=sqJ[0:n, :], op=ALU.mult)
            nc.scalar.activation(sqI[0:n, :], sqI[0:n, :], ACTF.Ln,
                                 bias=epsb[0:n, 0:1])
            nc.scalar.activation(sqI[0:n, :], sqI[0:n, :], ACTF.Exp,
                                 scale=-1.0)
            # crossN^2 in-place, then accumulate cc
            nc.scalar.square(crossN[0:n, :], crossN[0:n, :])
            nc.vector.scalar_tensor_tensor(
                out=P[0:n, :], in0=crossN[0:n, :], scalar=1.0,
                in1=sqI[0:n, :], op0=ALU.mult, op1=ALU.mult,
                accum_out=acc[0:n, COL_CC + c2:COL_CC + c2 + 1])

            # smoothness tiles 9..15 fill stage-2 troughs
            if c2 < 7:
                smooth_tile(9 + c2)

        # ---------------- final partition reduction ---------------------
        pF = psA.tile([128, 512], F32, tag="psA", name="pF")
        nc.tensor.matmul(pF[0:1, 0:NACC], ones_t[:], acc[:],
                         start=True, stop=True)
        outt = accp.tile([1, NACC], F32, tag="outt")
        nc.scalar.copy(outt[:], pF[0:1, 0:NACC])
        nc.sync.dma_start(part_d, outt[:])

    return
